# revision 1
# baseline (speedup 1.0000x reference)
import numpy as np
import jax
import jax.numpy as jnp

EPS = 1e-5  # torch BatchNorm2d default eps

B, C, H, W = 16, 256, 64, 64
N_CORES = 8


def _conv3x3(x, w, b):
    y = jax.lax.conv_general_dilated(
        x, w, (1, 1), 'SAME',
        dimension_numbers=('NCHW', 'OIHW', 'NCHW'))
    return y + b[None, :, None, None]


def _bn(x, g, b, m, v):
    inv = g * jax.lax.rsqrt(v + EPS)
    return x * inv[None, :, None, None] + (b - m * inv)[None, :, None, None]


def _forward(x, ec1_w, ec1_b, bn1_g, bn1_b, bn1_m, bn1_v,
             ec2_w, ec2_b, bn2_g, bn2_b, bn2_m, bn2_v,
             g1_w, g1_b, gbn_g, gbn_b, gbn_m, gbn_v,
             g2_w, g2_b, out_w, out_b):
    ef = jax.nn.relu(_bn(_conv3x3(x, ec1_w, ec1_b), bn1_g, bn1_b, bn1_m, bn1_v))
    ef = jax.nn.relu(_bn(_conv3x3(ef, ec2_w, ec2_b), bn2_g, bn2_b, bn2_m, bn2_v))

    x_pool = jnp.mean(x, axis=(2, 3))
    e_pool = jnp.mean(ef, axis=(2, 3))
    g = jnp.concatenate([x_pool, e_pool], axis=1)
    h = g @ g1_w.T + g1_b
    h = jax.nn.relu((h - gbn_m) * (gbn_g * jax.lax.rsqrt(gbn_v + EPS)) + gbn_b)
    gate = jax.nn.sigmoid(h @ g2_w.T + g2_b)

    edge_enh = jnp.einsum('bchw,oc->bohw', ef, out_w) + out_b[None, :, None, None]
    return x + gate[:, :, None, None] * edge_enh


_WEIGHT_KEYS = ('ec1_w', 'ec1_b', 'bn1_g', 'bn1_b', 'bn1_m', 'bn1_v',
                'ec2_w', 'ec2_b', 'bn2_g', 'bn2_b', 'bn2_m', 'bn2_v',
                'g1_w', 'g1_b', 'gbn_g', 'gbn_b', 'gbn_m', 'gbn_v',
                'g2_w', 'g2_b', 'out_w', 'out_b')

_pmapped = jax.pmap(
    _forward,
    in_axes=(0,) + (None,) * len(_WEIGHT_KEYS),
    devices=jax.devices()[:N_CORES])


def kernel(**inputs):
    x = np.asarray(inputs['x'], dtype=np.float32)
    # Data-parallel over batch: 16 samples -> 2 per core across 8 cores.
    xs = x.reshape(N_CORES, B // N_CORES, C, H, W)
    weights = [np.asarray(inputs[k], dtype=np.float32) for k in _WEIGHT_KEYS]
    out = _pmapped(xs, *weights)
    return np.asarray(out).reshape(B, C, H, W).astype(np.float32)

